# revision 9
# baseline (speedup 1.0000x reference)
"""Trainium2 Bass kernel for nn_NodeAttentionPerMetaPath (GAT-style node attention).

Reference computation (N=8192, F_IN=256, d=64):
    h      = x @ trans                      # [N, d]
    e1     = h @ attn[:d];  e2 = h @ attn[d:]
    scores = leaky_relu(e1 + e2.T, 0.2)     # [N, N]
    masked = where(mask==0, -1e15, scores)
    out    = softmax(masked, axis=1) @ h    # [N, d]

Design (v2 — collective-free, transpose-free):
  * exp(leaky(v)) = max(exp(v), exp(a*v)); with v = e1[r]+e2[j] and the
    per-row softmax invariances (drop exp(a*e1[r]), divide by C[r]):
        w''[j,r] = m[j,r] * B2[j] * max(D[j], invC[r])
    where D = exp(0.8*e2), invC = exp(-0.8*e1), B2 = exp(0.2*e2).
        out[r,:] = (P @ (B2*h)) / (P @ B2),  P[j,r] = m*max(D[j], invC[r])
  * Every core redundantly computes h/e2 for ALL N rows from a host-side
    x.T (fp16, 4MB) — removes both all-gathers and all cross-core skew.
  * The mask arrives host-transposed (and per-core row-permuted so "own"
    rows are always block 0), so all [N,N]-scale work happens directly in
    [j, r] layout: per 128-j chunk ONE tensor_scalar (4x fp16 DVE) + ONE
    mask tensor_tensor (2x DVE, some chunks on GpSimd) + two 512-moving
    accumulating matmuls. No PE transposes at all.
  * The [h | e2] trick: e2 = x @ (trans @ a2), so a single matmul with
    rhs = [trans | trans@a2] (65 cols) yields h and e2 together.
  * Output accumulates over all 64 j-chunks into 2 PSUM banks; the B2
    column of the augmented lhs yields the softmax denominator for free.
"""

from contextlib import ExitStack

import numpy as np

import concourse.bass as bass
import concourse.bacc as bacc
import concourse.mybir as mybir
import concourse.tile as tile
from concourse.bass_utils import run_bass_kernel_spmd

f32 = mybir.dt.float32
f16 = mybir.dt.float16
i32 = mybir.dt.int32
i8 = mybir.dt.int8

SDT = f16  # score-pipeline dtype
MDT = f16  # mask dtype on the wire (f16 -> 2x DVE TT; i8 -> half DMA, 1x TT)
MDT_NP = np.float16

Exp = mybir.ActivationFunctionType.Exp
MULT = mybir.AluOpType.mult
MAX = mybir.AluOpType.max
MIN = mybir.AluOpType.min

N_CORES = 8
N = 8192
F_IN = 256
D = 64
ALPHA = 0.2
R = N // N_CORES  # rows per core (1024)
NCH = N // 128  # j-chunks (64)
LAG = 8  # j-chunks of lag between phase-1 production and phase-2 consumption

# phase-2 mask-multiply split: every GP_EVERY-th chunk's TT runs on GpSimd
GP_EVERY = 3


def build_kernel(ctx: ExitStack, tc: tile.TileContext, xt, transp, transt, a12, maskt, outT):
    nc = tc.nc

    singles = ctx.enter_context(tc.tile_pool(name="singles", bufs=1))
    maskp = ctx.enter_context(tc.tile_pool(name="maskp", bufs=6))
    vp = ctx.enter_context(tc.tile_pool(name="vp", bufs=3))

    # ---------------- DMA issue (order drives queue order) ----------------
    # x.T pieces on the scalar-engine HWDGE queue: 8 x [128, 2, 1024] f16
    xt_tiles = []
    for p in range(8):
        t = singles.tile([128, 2, 1024], f16, tag=f"xt{p}")
        nc.scalar.dma_start(out=t, in_=xt[p])
        xt_tiles.append(t)

    trans_sb = singles.tile([128, 2, D], f16, tag="trans")
    nc.gpsimd.dma_start(out=trans_sb, in_=transp.rearrange("(c p) d -> p c d", p=128))
    transT_sb = singles.tile([D, F_IN], f16, tag="transT")
    nc.gpsimd.dma_start(out=transT_sb, in_=transt[:, :])
    a12_sb = singles.tile([D, 2], f16, tag="a12")
    nc.gpsimd.dma_start(out=a12_sb, in_=a12[:, :])

    # mask group stream on the sync-engine queue: 16 x 1MB (4 j-chunks each),
    # 8KB contiguous per partition (paced by tile-slot release)
    mask_tiles = []
    for g in range(NCH // 4):
        m = maskp.tile([128, 4, R], MDT, tag="m", bufs=3)
        nc.sync.dma_start(out=m, in_=maskt[g])
        mask_tiles.append(m)

    # ---------------- persistent sbuf ----------------
    haug = singles.tile([128, NCH, D + 1], SDT, tag="haug")  # [j%128, jc, B2*h | B2]
    d_all = singles.tile([128, NCH], f32, tag="d_all")
    b2_all = singles.tile([128, NCH], f32, tag="b2")
    invc_rep = singles.tile([128, R], SDT, tag="invc")
    invc_row = singles.tile([1, R], SDT, tag="invcr")
    ones128 = singles.tile([1, 128], f16, tag="ones128")
    nc.vector.memset(ones128, 1.0)
    ones64 = singles.tile([1, D], f32, tag="ones64")
    nc.vector.memset(ones64, 1.0)
    rhs2 = singles.tile([128, 2, D + 1], f16, tag="rhs2")  # [trans | trans@a2]
    ta12 = singles.tile([128, 2, 2], f16, tag="ta12")  # trans@a1 | trans@a2

    # ---------------- preamble: ta12, rhs2, e1 -> invC ----------------
    with tc.tile_pool(name="ps_pre", bufs=3, space="PSUM") as ps_pre:
        for fc in range(2):
            pst = ps_pre.tile([128, 512], f32, tag="pre")
            nc.tensor.matmul(
                pst[:, 0:2],
                transT_sb[:, fc * 128 : (fc + 1) * 128],
                a12_sb,
                start=True,
                stop=True,
            )
            nc.vector.tensor_copy(ta12[:, fc, :], pst[:, 0:2])
        nc.vector.tensor_copy(rhs2[:, :, 0:D], trans_sb)
        for fc in range(2):
            nc.vector.tensor_copy(rhs2[:, fc, D : D + 1], ta12[:, fc, 1:2])

        # e1 for own rows (piece 0 after host permutation) -> invC
        for h2 in range(2):
            pse = ps_pre.tile([128, 512], f32, tag="pre")
            for fc in range(2):
                nc.tensor.matmul(
                    pse[0:1, :],
                    ta12[:, fc, 0:1],
                    xt_tiles[0][:, fc, h2 * 512 : (h2 + 1) * 512],
                    start=(fc == 0),
                    stop=(fc == 1),
                )
            nc.scalar.activation(
                invc_row[0:1, h2 * 512 : (h2 + 1) * 512],
                pse[0:1, :],
                Exp,
                scale=-(1.0 - ALPHA),
            )
        # broadcast invC over all 128 partitions via K=1 matmul
        for h2 in range(2):
            psb = ps_pre.tile([128, 512], f32, tag="pre")
            nc.tensor.matmul(
                psb,
                ones128,
                invc_row[0:1, h2 * 512 : (h2 + 1) * 512],
                start=True,
                stop=True,
            )
            nc.vector.tensor_copy(invc_rep[:, h2 * 512 : (h2 + 1) * 512], psb)

    heps = ctx.enter_context(tc.tile_pool(name="heps", bufs=4, space="PSUM"))
    ps_acc = ctx.enter_context(tc.tile_pool(name="ps_acc", bufs=1, space="PSUM"))
    po0 = ps_acc.tile([D + 1, 512], f32, tag="po0")
    po1 = ps_acc.tile([D + 1, 512], f32, tag="po1")

    # ---------------- fused phase-1/phase-2 pipeline ----------------
    def phase1_step(k):
        piece = xt_tiles[k // 8]
        sub = k % 8
        ps_he = heps.tile([128, D + 1], f32, tag="he", bufs=4)
        for fc in range(2):
            nc.tensor.matmul(
                ps_he,
                piece[:, fc, sub * 128 : (sub + 1) * 128],
                rhs2[:, fc, :],
                start=(fc == 0),
                stop=(fc == 1),
            )
        nc.scalar.copy(haug[:, k, :], ps_he)  # f32 -> f16, e2 still in col D

    def piece_exps(p):
        sl = slice(p * 8, p * 8 + 8)
        nc.scalar.activation(d_all[:, sl], haug[:, sl, D], Exp, scale=1.0 - ALPHA)
        nc.scalar.activation(b2_all[:, sl], haug[:, sl, D], Exp, scale=ALPHA)
        for j in range(p * 8, p * 8 + 8):
            nc.scalar.mul(haug[:, j, 0:D], haug[:, j, 0:D], b2_all[:, j : j + 1])
        nc.scalar.copy(haug[:, sl, D], b2_all[:, sl])

    def phase2_group(g):
        # p = max(invC, D) * m
        mg = mask_tiles[g]
        vg = vp.tile([128, 4, R], SDT, tag="v", bufs=3)
        if g % 2 == 1 and g < 15:  # GpSimd group: 4 DVE TS + whole-group GpSimd mult
            for i in range(4):
                kk = g * 4 + i
                nc.vector.tensor_scalar(
                    vg[:, i, :], invc_rep, d_all[:, kk : kk + 1], None, MAX
                )
            nc.gpsimd.tensor_tensor(vg, vg, mg, MULT)
        else:  # DVE group: fused (invC max D) min m per chunk
            for i in range(4):
                kk = g * 4 + i
                nc.vector.scalar_tensor_tensor(
                    vg[:, i, :], invc_rep, d_all[:, kk : kk + 1], mg[:, i, :], MAX, MULT
                )
        for i in range(4):
            kk = g * 4 + i
            nc.tensor.matmul(
                po0, haug[:, kk, :], vg[:, i, 0:512], start=(kk == 0), stop=(kk == NCH - 1)
            )
            nc.tensor.matmul(
                po1, haug[:, kk, :], vg[:, i, 512:], start=(kk == 0), stop=(kk == NCH - 1)
            )

    NG = NCH // 4
    GLAG = LAG // 4
    for step in range(NG + GLAG):
        if step < NG:
            for i in range(4):
                k = step * 4 + i
                phase1_step(k)
                if k % 8 == 7:
                    piece_exps(k // 8)
        if step >= GLAG:
            phase2_group(step - GLAG)

    # ---------------- normalize + store ----------------
    outp = ctx.enter_context(tc.tile_pool(name="outp", bufs=1))
    for h2, po in enumerate((po0, po1)):
        den = outp.tile([1, 512], f32, tag="den", bufs=2)
        nc.vector.tensor_copy(den, po[D : D + 1, :])
        recip = outp.tile([1, 512], f32, tag="rc", bufs=2)
        nc.vector.reciprocal_approx_fast(recip, den)
        rr = ps_acc.tile([D, 512], f32, tag="rr", bufs=2)
        nc.tensor.matmul(rr, ones64, recip, start=True, stop=True)
        rr_sb = outp.tile([D, 512], f32, tag="rrsb", bufs=2)
        nc.scalar.copy(rr_sb, rr)
        o_t = outp.tile([D, 512], f32, tag="ot", bufs=2)
        nc.vector.tensor_tensor(o_t, po[0:D, :], rr_sb, MULT)
        nc.gpsimd.dma_start(out=outT[:, h2 * 512 : (h2 + 1) * 512], in_=o_t)


def build_nc():
    nc = bacc.Bacc("TRN2", num_devices=N_CORES)
    xt = nc.dram_tensor("xt", [8, 128, 2, 1024], f16, kind="ExternalInput")
    transp = nc.dram_tensor("transp", [F_IN, D], f16, kind="ExternalInput")
    transt = nc.dram_tensor("transt", [D, F_IN], f16, kind="ExternalInput")
    a12 = nc.dram_tensor("a12", [D, 2], f16, kind="ExternalInput")
    maskt = nc.dram_tensor("maskt", [NCH // 4, 128, 4, R], MDT, kind="ExternalInput")
    outT = nc.dram_tensor("outT", [D, R], f32, kind="ExternalOutput")
    with ExitStack() as ctx:
        tc = ctx.enter_context(tile.TileContext(nc))
        build_kernel(
            ctx, tc, xt[:, :, :, :], transp[:, :], transt[:, :], a12[:, :],
            maskt[:, :, :, :], outT[:, :]
        )
    nc.compile()
    return nc


LAST_RESULTS = None


def kernel(x, mask, trans, attn, _trace=False):
    x = np.asarray(x, dtype=np.float32)
    mask = np.asarray(mask)
    trans = np.asarray(trans, dtype=np.float32)
    attn = np.asarray(attn, dtype=np.float32)

    xt16 = np.ascontiguousarray(x.T.astype(np.float16))
    transp16 = np.ascontiguousarray(trans.astype(np.float16))
    transt16 = np.ascontiguousarray(trans.T.astype(np.float16))
    a12h = np.ascontiguousarray(
        np.concatenate([attn[:D], attn[D:]], axis=1).astype(np.float16)
    )

    nc = build_nc()
    in_maps = []
    for c in range(N_CORES):
        perm = np.r_[c * R : (c + 1) * R, 0 : c * R, (c + 1) * R : N]
        mT = mask[c * R : (c + 1) * R, :].T[perm].astype(MDT_NP)
        mT4 = np.ascontiguousarray(
            mT.reshape(NCH // 4, 4, 128, R).transpose(0, 2, 1, 3)
        )
        in_maps.append(
            {
                "xt": np.ascontiguousarray(
                    xt16[:, perm].reshape(2, 128, 8, 1024).transpose(2, 1, 0, 3)
                ),
                "transp": transp16,
                "transt": transt16,
                "a12": a12h,
                "maskt": mT4,
            }
        )
    res = run_bass_kernel_spmd(nc, in_maps, list(range(N_CORES)), trace=_trace)
    global LAST_RESULTS
    LAST_RESULTS = res
    out = np.concatenate([res.results[c]["outT"].T for c in range(N_CORES)], axis=0)
    return np.ascontiguousarray(out, dtype=np.float32)


if __name__ == "__main__":
    nc = build_nc()
    print("built OK")
